# revision 12
# baseline (speedup 1.0000x reference)
"""Trainium2 Bass kernel for the ragged-sequence cross-attention module.

Math (reference):
    f       = Wf @ f_pre_in.T + bf                      (H, M)
    b_feat  = Wb @ b_pre_in[g] + bb                     per graph (H, N)
    bv_feat = Wbv @ bv_in[g] + bbv                      per graph (H, N)
    w_euc   = softmax((b_feat[g,:32].T @ f[:32]) / 8)   per node over N
    w_geo   = softmax((b_feat[g,32:].T @ f[32:]) / 8)
    out     = [bv_feat[g] @ w_euc, bv_feat[g] @ w_geo] @ Wo.T + bo   (M, H)

Algebraic folds (host-side weight preprocessing only):
  * bb never enters: softmax over n is invariant to per-node constants.
  * Wbv/bbv/Wo/bo fold into A = Wo[:, :H] @ Wbv, Bm = Wo[:, H:] @ Wbv and
    bo_tot = bo + (Wo[:, :H] + Wo[:, H:]) @ bbv, since sum(softmax) == 1.
    The device applies raw bv (6 channels) + a ones channel (denominator)
    and projects [e/de; 1; g/dg; 1] with a (14 -> 64) matmul.

Sharding: one core per graph (B == n_cores == 8).  Nodes are sorted by
graph, so core g takes graph g's contiguous node range, padded to
NT = ceil(maxL/128) m-tiles of 128 nodes.

Device pipeline, v3.  Engine facts this build enforces: Pool/GPSIMD cannot
touch PSUM, and any op may read at most ONE PSUM operand.  So the PSUM-exit
work (exps, feature/psa/fin copies) is split across ACT and DVE, and Pool
handles all DMAs plus SBUF-only math:
  scores  (128n, 4, 128m) per (tile, half) -> PSUM ring of 5
  exp     ACT true Exp -> fp8e4, or DVE Schraudolph affine map to fp8e4
          bits (byte = s*log2(e) + 56.86; at fp8 output precision this
          costs ~0.05% extra output error)
  apply   fp8 DoubleRow matmuls, lhsT = [bv(6) | ones...] -> psa rows 0..6
          = raw outs + denominator, rows 32..38 = denominator x7
  norm    copy psa rows 0:39 -> SBUF (ACT/DVE), then ONE Pool divide
          cat = rows 0:7 / rows 32:39  (SBUF-only, so Pool is legal)
  fin     (14 -> 64) bf16 matmul -> PSUM -> ACT/DVE copy -> DMA out
The remainder tile is processed mid-stream with ops sliced to its ~16 real
columns (stale te/cat bytes beyond that are second-use ring buffers, hence
finite, and feed only discarded slots); a full tile runs last with its
h0-side apply/norm/fin pre-issued so the post-last-exp chain is short.
"""

import sys

for _p in ("/opt/trn_rl_repo", "/root/.axon_site/_ro/trn_rl_repo"):
    if _p not in sys.path:
        sys.path.append(_p)

import numpy as np

import bass_rust

import concourse.bass as bass
import concourse.mybir as mybir
from concourse.bass_utils import run_bass_kernel_spmd
from concourse.tile import TileContext
from concourse.vector_clock import ScopedClock, VectorClock

F32 = mybir.dt.float32
BF16 = mybir.dt.bfloat16
FP8 = mybir.dt.float8e4
U8 = mybir.dt.uint8

NP_BF16 = mybir.dt.np(BF16)
NP_FP8 = mybir.dt.np(FP8)

# Problem shapes (hardcoded per the harness contract).
M, B, N, FD, BD, BVD, H = 4096, 8, 512, 128, 128, 6, 64
H2 = H // 2
N_CORES = 8

# fp8e4(v) bits = 8*(log2(v)+7) with mantissa-linear interpolation; the +0.36
# centers the 2^frac-1 vs frac error band; +0.5 assumes truncating f32->u8.
SCHRAUDOLPH_MUL = 1.4426950408889634  # log2(e) (the /8 score scale and the
SCHRAUDOLPH_ADD = 56.5 + 0.36  # x8 bits-per-octave cancel)

# The walrus build in this environment rejects multiple semaphore waits
# on one instruction; carry every wait on its own nop.
_MAX_WAITS = 1


class _ChunkedDrainTileContext(TileContext):
    """The walrus build in this environment rejects >1 semaphore wait on a
    single instruction, which breaks TileContext's final drain (it carries
    one wait per touched proc).  Split those waits across one nop per proc;
    each engine executes serially, so a bare drain afterwards is
    equivalent."""

    _nop_uid = 0

    def _add_instruction(self, inst):
        si = inst.sync_info
        if (
            si is not None
            and si.on_wait
            and len(si.on_wait) > _MAX_WAITS
            and inst.engine != mybir.EngineType.Unassigned
        ):
            waits = list(si.on_wait)
            excess, keep = waits[:-_MAX_WAITS], waits[-_MAX_WAITS:]
            for i in range(0, len(excess), _MAX_WAITS):
                _ChunkedDrainTileContext._nop_uid += 1
                nop = mybir.InstNoOp(
                    name=f"splitw{_ChunkedDrainTileContext._nop_uid}", ins=[], outs=[]
                )
                nop.engine = inst.engine
                nop.sync_info = bass_rust.SyncInfo(
                    on_wait=excess[i : i + _MAX_WAITS], on_update=[]
                )
                super()._add_instruction(nop)
            inst.sync_info = bass_rust.SyncInfo(on_wait=keep, on_update=si.on_update)
        super()._add_instruction(inst)

    def _drain_and_barrier(self, tick_clock, wait_clock):
        nc = self.nc
        g = tick_clock.global_clock
        nprocs = len(g)
        for i in range(nprocs):
            if g[i] > 0:
                vc = VectorClock([0] * nprocs)
                vc.require_at_least(i, g[i])
                nop_inst = nc.sync.nop(nofuse=True, hint=f"drain_wait_p{i}")
                wait_clock.add_sem_waits(nop_inst.ins, ScopedClock({None: vc}))
        nc.sync.drain()
        nc.all_engine_barrier()
        assert self.sems is not None
        popped = nc._tile_sem_poison_stack.pop()
        assert popped is self._sem_poison
        nc.clear_and_free_semaphores(list(self.sems.allocated().values()))
        nc.all_engine_barrier()


def _n_tiles(maxL):
    return max(1, -(-maxL // 128))


def _plan(NT, maxL):
    """Per-tile schedule: processing order, exp engines (h0, h1), psa-copy
    engine, fin-copy engine, exp/psa slice width.  'A' = ACT, 'D' = DVE.
    The remainder tile runs second-to-last (its sliced ops are cheap); the
    last full tile's tail is half-pipelined."""
    wr = maxL - 128 * (NT - 1)
    ew = [128] * NT
    if 0 < wr < 128:
        ew[NT - 1] = min(128, -(-wr // 16) * 16)
    if NT >= 3:
        order = list(range(NT - 2)) + [NT - 1, NT - 2]
    else:
        order = list(range(NT))
    exp_eng = {}
    for t in range(NT):
        exp_eng[(t, 0)] = "A"
        exp_eng[(t, 1)] = "D" if (t in (0, 1) or ew[t] < 128) else "A"
    cp_eng = {t: ("A" if i % 2 == 0 else "D") for i, t in enumerate(order)}
    fc_eng = {t: ("D" if i % 2 == 0 else "A") for i, t in enumerate(order)}
    return order, exp_eng, cp_eng, fc_eng, ew


def build_program(maxL, reps=1):
    """Build the per-core SPMD Bass program (identical on all 8 cores)."""
    nc = bass.Bass()
    NT = _n_tiles(maxL)
    TM = 128 * NT
    W = 128
    order, exp_eng, cp_eng, fc_eng, ew = _plan(NT, maxL)
    last_t = order[-1]

    d_pk = nc.declare_dram_parameter("pk", [128, 392], BF16, isOutput=False)
    d_bp = nc.declare_dram_parameter("bp", [BD, N], BF16, isOutput=False)
    d_ft = nc.declare_dram_parameter("ft", [FD, TM], BF16, isOutput=False)
    d_out = nc.declare_dram_parameter("outT", [reps, H, TM], BF16, isOutput=True)

    DR = mybir.MatmulPerfMode.DoubleRow
    Exp = mybir.ActivationFunctionType.Exp
    Copy = mybir.ActivationFunctionType.Copy
    Div = mybir.AluOpType.divide
    Mult = mybir.AluOpType.mult
    Add = mybir.AluOpType.add

    # finA column slot per non-last tile (packed in processing order of
    # sorted tile index so out DMA pieces are contiguous in d_out).
    nonlast = sorted(t for t in order if t != last_t)
    fslot = {t: 128 * i for i, t in enumerate(nonlast)}

    with _ChunkedDrainTileContext(nc) as tc, nc.allow_low_precision(
        reason="bf16/fp8 rounding of fp32 data"
    ):
        with (
            tc.tile_pool(name="const", bufs=1) as cp,
            tc.tile_pool(name="work", bufs=2) as wkp,
            tc.tile_pool(name="te", bufs=6) as tep,
            tc.tile_pool(name="ps_s", bufs=4, space="PSUM") as pss,
            tc.tile_pool(name="ps_a", bufs=2, space="PSUM") as psap,
        ):
            # Preload the ACT Exp table at t=0 (a real first Exp would pay
            # the ~1.3us table load on the critical path).
            t_warm = cp.tile([1, 1], F32, tag="warm")
            nc.vector.memset(t_warm[:], 0.0)
            t_warm2 = cp.tile([1, 1], F32, tag="warm2")
            nc.scalar.activation(t_warm2[:], t_warm[:], Exp)

            # Input DMAs: packed weights + ft piece 0 on the SP HWDGE queue;
            # b_pre halves + remaining ft columns on the Pool SWDGE queue
            # (which bypasses the shared HWDGE block).
            t_pk = cp.tile([128, 392], BF16, tag="pk")
            nc.sync.dma_start(t_pk[:], d_pk[:])
            t_ft = cp.tile([FD, TM], BF16, tag="ft")
            nc.sync.dma_start(t_ft[:, 0:128], d_ft[:, 0:128])
            t_bp = cp.tile([BD, N], BF16, tag="bp")
            nc.gpsimd.dma_start(t_bp[:, 0:256], d_bp[:, 0:256])
            nc.gpsimd.dma_start(t_bp[:, 256:512], d_bp[:, 256:512])
            if TM > 128:
                nc.gpsimd.dma_start(t_ft[:, 128:TM], d_ft[:, 128:TM])

            # Views into the packed weights.
            t_wft = t_pk[:, 0:64]
            t_wbt = t_pk[:, 64:128]
            t_wfe = t_pk[0:7, 128:192]
            t_wfg = t_pk[0:7, 192:256]
            t_bf = t_pk[0:64, 256:258].bitcast(F32)
            t_bv = t_pk[:, 258:386].bitcast(FP8).rearrange("p (j c) -> p j c", j=4)

            for rep in range(reps):
                # ---- boundary features: two (128 -> 64, 256) matmuls; the
                # PSUM->SBUF copies ride ACT (first half) and DVE ----
                ps_bh = psap.tile([H, N], F32, tag="finA", bufs=1)
                nc.tensor.matmul(
                    ps_bh[:, 0:256], t_wbt, t_bp[:, 0:256], start=True, stop=True
                )
                nc.tensor.matmul(
                    ps_bh[:, 256:512], t_wbt, t_bp[:, 256:512], start=True, stop=True
                )
                t_bh = wkp.tile([H, N], BF16, tag="bh")
                nc.scalar.copy(t_bh[:, 0:256], ps_bh[:, 0:256])
                nc.scalar.copy(t_bh[:, 256:512], ps_bh[:, 256:512])

                # ---- node features: piece 0 early (gates first scores);
                # all bias-add copies ride DVE ahead of its first exp ----
                t_fh = wkp.tile([H, TM], BF16, tag="fh")
                ps_f0 = psap.tile([H, 128], F32, tag="finB", bufs=1)
                nc.tensor.matmul(ps_f0[:], t_wft, t_ft[:, 0:128], start=True, stop=True)
                nc.vector.tensor_scalar_add(t_fh[:, 0:128], ps_f0[:], t_bf)
                ps_fr = []
                fr_pieces = []
                for i in range(1, NT, 2):
                    j = min(i + 2, NT)
                    p = psap.tile([H, 128 * (j - i)], F32, tag="a")
                    nc.tensor.matmul(
                        p[:], t_wft, t_ft[:, 128 * i : 128 * j],
                        start=True, stop=True,
                    )
                    ps_fr.append(p)
                    fr_pieces.append((i, j))
                # Bias-adds for the remaining pieces go on DVE ahead of its
                # first exp (they are ready long before the exp's scores).
                for p, (i, j) in zip(ps_fr, fr_pieces):
                    nc.vector.tensor_scalar_add(
                        t_fh[:, 128 * i : 128 * j], p[:], t_bf
                    )

                def bh_ap(hi, j):  # noqa: B023
                    return t_bh[H2 * hi : H2 * hi + H2, 128 * j : 128 * (j + 1)]

                def fh_ap(hi, t):  # noqa: B023
                    return t_fh[H2 * hi : H2 * hi + H2, 128 * t : 128 * (t + 1)]

                ps_finA = psap.tile(
                    [H, 128 * max(1, NT - 1)], F32, tag="finA", bufs=1
                )
                ps_finB = psap.tile([H, 128], F32, tag="finB", bufs=1)
                t_fout = wkp.tile([H, TM], BF16, tag="fout")

                te_tiles = {}
                psa_tiles = {}
                sb_tiles = {}
                cat_tiles = {}

                def emit_scores(t, hi):  # noqa: B023
                    ps_s = pss.tile([128, 4, W], F32, tag="s")
                    for j in range(4):
                        nc.tensor.matmul(
                            ps_s[:, j, :], bh_ap(hi, j), fh_ap(hi, t),
                            start=True, stop=True,
                        )
                    return ps_s

                def emit_exp(t, hi, ps_s):  # noqa: B023
                    te = tep.tile([128, 4, W], FP8, tag="te")
                    e = ew[t]
                    if exp_eng[(t, hi)] == "A":
                        nc.scalar.activation(
                            te[:, :, 0:e], ps_s[:, :, 0:e], Exp, scale=0.125
                        )
                    else:
                        nc.vector.tensor_scalar(
                            te[:, :, 0:e].bitcast(U8), ps_s[:, :, 0:e],
                            SCHRAUDOLPH_MUL, SCHRAUDOLPH_ADD, Mult, Add,
                        )
                    te_tiles[(t, hi)] = te

                def emit_apply(t, hi=None):  # noqa: B023
                    # hi=None: both halves; else just that half's 2 DR mms.
                    if t not in psa_tiles:
                        psa_tiles[t] = psap.tile([64, 2, W], F32, tag="a", name=f"psa{t}")
                    t_psa = psa_tiles[t]
                    his = range(2) if hi is None else [hi]
                    for h in his:
                        te = te_tiles[(t, h)]
                        for jp in range(2):
                            nc.tensor.matmul(
                                t_psa[0:64, h, :],
                                t_bv[:, 2 * jp : 2 * jp + 2, :],
                                te[:, 2 * jp : 2 * jp + 2, :],
                                start=(jp == 0), stop=(jp == 1),
                                perf_mode=DR,
                            )

                def emit_norm(t, hi=None):  # noqa: B023
                    # Copy the replicated denominator rows 32:39 to SBUF
                    # (partition-aligned) on ACT/DVE, then one DVE divide:
                    # cat = psa rows 0:7 (the single legal PSUM operand) /
                    # the SBUF denom at quadrant-aligned base 32.
                    t_psa = psa_tiles[t]
                    e = ew[t]
                    if t not in sb_tiles:
                        sb_tiles[t] = wkp.tile([39, 2, W], F32, tag="psb", name=f"psb{t}")
                        cat_tiles[t] = wkp.tile([64, 2, W], BF16, tag="cat", name=f"cat{t}")
                    t_sb = sb_tiles[t]
                    his = slice(None) if hi is None else slice(hi, hi + 1)
                    nc.vector.reciprocal(
                        t_sb[32:39, his, 0:e], t_psa[32:39, his, 0:e]
                    )
                    if hi is None or hi == 1:
                        t_cat = cat_tiles[t]
                        nc.vector.tensor_mul(
                            t_cat[0:7, :, 0:e], t_psa[0:7, :, 0:e],
                            t_sb[32:39, :, 0:e],
                        )

                def emit_fin(t):  # noqa: B023
                    t_cat = cat_tiles[t]
                    if t == last_t:
                        fin = ps_finB[:, 0:W]
                    else:
                        fin = ps_finA[:, fslot[t] : fslot[t] + W]
                    nc.tensor.matmul(
                        fin, t_wfe, t_cat[0:7, 0, :], start=True, stop=False
                    )
                    nc.tensor.matmul(
                        fin, t_wfg, t_cat[0:7, 1, :], start=False, stop=True
                    )

                def emit_fin_copy(t):  # noqa: B023
                    # fin PSUM -> t_fout columns (sliced to real width).
                    e = ew[t]
                    src = (
                        ps_finB[:, 0:e]
                        if t == last_t
                        else ps_finA[:, fslot[t] : fslot[t] + e]
                    )
                    dst = t_fout[:, 128 * t : 128 * t + e]
                    if fc_eng[t] == "A":
                        nc.scalar.copy(dst, src)
                    else:
                        nc.vector.tensor_copy(dst, src)

                # --- main pipeline over processing order ---
                # pos i emits scores+exps(i); apply/norm trail by 1, fin by 2.
                def tail_emit(i):  # noqa: B023
                    if i >= 1:
                        emit_apply(order[i - 1])
                        emit_norm(order[i - 1])
                    if i >= 2:
                        emit_fin(order[i - 2])
                        emit_fin_copy(order[i - 2])

                for i, t in enumerate(order[:-1]):
                    for hi in range(2):
                        ps_s = emit_scores(t, hi)
                        emit_exp(t, hi, ps_s)
                    tail_emit(i)
                # Last tile: h0's exp/apply/norm-copy and even its fin start=
                # half are issued while h1's exp runs, so the post-last-exp
                # chain is apply(h1) -> copy(h1) -> divide -> fin -> copy.
                i = len(order) - 1
                ps_s = emit_scores(last_t, 0)
                emit_exp(last_t, 0, ps_s)
                ps_s = emit_scores(last_t, 1)
                tail_emit(i)
                emit_apply(last_t, hi=0)
                emit_norm(last_t, hi=0)
                emit_exp(last_t, 1, ps_s)
                emit_apply(last_t, hi=1)
                emit_norm(last_t, hi=1)
                if i >= 1:
                    emit_fin(order[i - 1])
                    emit_fin_copy(order[i - 1])
                emit_fin(last_t)
                emit_fin_copy(last_t)

                # --- out DMAs: contiguous runs of non-last tiles on SP,
                # the last tile's piece on Pool SWDGE ---
                runs = []
                for t in nonlast:
                    if runs and runs[-1][1] == t:
                        runs[-1][1] = t + 1
                    else:
                        runs.append([t, t + 1])
                for t0, t1 in runs:
                    e = 128 * (t1 - 1 - t0) + ew[t1 - 1]
                    nc.sync.dma_start(
                        d_out[rep][:, 128 * t0 : 128 * t0 + e],
                        t_fout[:, 128 * t0 : 128 * t0 + e],
                    )
                nc.gpsimd.dma_start(
                    d_out[rep][:, 128 * last_t : 128 * last_t + ew[last_t]],
                    t_fout[:, 128 * last_t : 128 * last_t + ew[last_t]],
                )

    return nc


def stage_inputs(inputs, bounds, maxL):
    """Build the 8 per-core input maps from the full problem inputs."""
    TM = 128 * _n_tiles(maxL)
    f_pre_in = np.asarray(inputs["f_pre_in"], dtype=np.float32)
    b_pre_in = np.asarray(inputs["b_pre_in"], dtype=np.float32)
    bv_in = np.asarray(inputs["bv_in"], dtype=np.float32)
    Wf = np.asarray(inputs["Wf"], dtype=np.float32)
    bf = np.asarray(inputs["bf"], dtype=np.float32)
    Wb = np.asarray(inputs["Wb"], dtype=np.float32)
    Wbv = np.asarray(inputs["Wbv"], dtype=np.float32)
    bbv = np.asarray(inputs["bbv"], dtype=np.float32)
    Wo = np.asarray(inputs["Wo"], dtype=np.float32)
    bo = np.asarray(inputs["bo"], dtype=np.float32)

    # Folded output weights: out = A @ (bv@w_e)/de + Bm @ (bv@w_g)/dg + bo_tot
    A = Wo[:, :H] @ Wbv
    Bm = Wo[:, H:] @ Wbv
    bo_tot = bo + (Wo[:, :H] + Wo[:, H:]) @ bbv

    wpk = np.zeros((128, 256), np.float32)
    wpk[:, 0:64] = Wf.T
    wpk[:, 64:128] = Wb.T
    # final lhsT: rows 0-5 = A cols, 6 = bo_tot, same for Bm (geo)
    wpk[0:6, 128:192] = A.T
    wpk[6, 128:192] = bo_tot
    wpk[0:6, 192:256] = Bm.T

    in_maps = []
    for c in range(N_CORES):
        s, e = int(bounds[c]), int(bounds[c + 1])
        L = e - s
        ft = np.zeros((FD, TM), np.float32)
        ft[:, :L] = f_pre_in[s:e].T
        bv8 = np.zeros((128, 4, 64), np.float32)
        for j in range(4):
            bv8[:, j, 0:BVD] = bv_in[c, :, 128 * j : 128 * (j + 1)].T
        bv8[:, :, 6] = 1.0       # -> psa row 6 = denominator (cat ones row)
        bv8[:, :, 32:39] = 1.0   # -> psa rows 32-38 = denominator block
        # Packed consts: wpk bf16 | bf f32 (rows 0:64) | bv8 fp8, as bytes.
        pk = np.zeros((128, 784), np.uint8)
        pk[:, 0:512] = wpk.astype(NP_BF16).view(np.uint8)
        pk[0:64, 512:516] = bf.astype(np.float32).view(np.uint8).reshape(64, 4)
        pk[:, 516:772] = bv8.astype(NP_FP8).reshape(128, 256).view(np.uint8)
        in_maps.append(
            {
                "pk": pk.view(NP_BF16),
                "ft": ft.astype(NP_BF16),
                "bp": b_pre_in[c].astype(NP_BF16),
            }
        )
    return in_maps


def unstage_output(results, bounds, maxL):
    out = np.zeros((M, H), np.float32)
    for c in range(N_CORES):
        s, e = int(bounds[c]), int(bounds[c + 1])
        L = e - s
        outT = results[c]["outT"][0]
        out[s:e] = outT[:, :L].T.astype(np.float32)
    return out


_NC_CACHE = {}


def _program(maxL, reps=1):
    key = (maxL, reps)
    if key not in _NC_CACHE:
        _NC_CACHE[key] = build_program(maxL, reps=reps)
    return _NC_CACHE[key]


def kernel(**inputs):
    assert np.asarray(inputs["f_pre_in"]).shape == (M, FD)
    batch = np.asarray(inputs["f_pre_batch"]).astype(np.int64)
    bounds = np.searchsorted(batch, np.arange(B + 1))
    maxL = int(np.diff(bounds).max())
    in_maps = stage_inputs(inputs, bounds, maxL)
    nc = _program(maxL)
    res = run_bass_kernel_spmd(nc, in_maps, core_ids=list(range(N_CORES)))
    return unstage_output(res.results, bounds, maxL)


if __name__ == "__main__":
    rng = np.random.default_rng(0)
    demo = {
        "f_pre_in": rng.standard_normal((M, FD), dtype=np.float32),
        "f_pre_batch": np.sort(rng.integers(0, B, size=M)),
        "b_pre_in": rng.standard_normal((B, BD, N), dtype=np.float32),
        "bv_in": rng.standard_normal((B, BVD, N), dtype=np.float32),
        "Wf": rng.standard_normal((H, FD), dtype=np.float32) * 0.05,
        "bf": rng.standard_normal(H, dtype=np.float32) * 0.05,
        "Wb": rng.standard_normal((H, BD), dtype=np.float32) * 0.05,
        "bb": rng.standard_normal(H, dtype=np.float32) * 0.05,
        "Wbv": rng.standard_normal((H, BVD), dtype=np.float32) * 0.05,
        "bbv": rng.standard_normal(H, dtype=np.float32) * 0.05,
        "Wo": rng.standard_normal((H, 2 * H), dtype=np.float32) * 0.05,
        "bo": rng.standard_normal(H, dtype=np.float32) * 0.05,
    }
    out = kernel(**demo)
    print("kernel output", out.shape, out.dtype, float(np.abs(out).mean()))


# revision 14
# speedup vs baseline: 1.0391x; 1.0391x over previous
"""Trainium2 Bass kernel for the ragged-sequence cross-attention module.

Math (reference):
    f       = Wf @ f_pre_in.T + bf                      (H, M)
    b_feat  = Wb @ b_pre_in[g] + bb                     per graph (H, N)
    bv_feat = Wbv @ bv_in[g] + bbv                      per graph (H, N)
    w_euc   = softmax((b_feat[g,:32].T @ f[:32]) / 8)   per node over N
    w_geo   = softmax((b_feat[g,32:].T @ f[32:]) / 8)
    out     = [bv_feat[g] @ w_euc, bv_feat[g] @ w_geo] @ Wo.T + bo   (M, H)

Algebraic folds (host-side weight preprocessing only):
  * bb never enters: softmax over n is invariant to per-node constants.
  * Wbv/bbv/Wo/bo fold into A = Wo[:, :H] @ Wbv, Bm = Wo[:, H:] @ Wbv and
    bo_tot = bo + (Wo[:, :H] + Wo[:, H:]) @ bbv, since sum(softmax) == 1.
    The device applies raw bv (6 channels) + a ones channel (denominator)
    and projects [e/de; 1; g/dg; 1] with a (14 -> 64) matmul.

Sharding: one core per graph (B == n_cores == 8).  Nodes are sorted by
graph, so core g takes graph g's contiguous node range, padded to
NT = ceil(maxL/128) m-tiles of 128 nodes.

Device pipeline, v3.  Engine facts this build enforces: Pool/GPSIMD cannot
touch PSUM, and any op may read at most ONE PSUM operand.  So the PSUM-exit
work (exps, feature/psa/fin copies) is split across ACT and DVE, and Pool
handles all DMAs plus SBUF-only math:
  scores  (128n, 4, 128m) per (tile, half) -> PSUM ring of 5
  exp     ACT true Exp -> fp8e4, or DVE Schraudolph affine map to fp8e4
          bits (byte = s*log2(e) + 56.86; at fp8 output precision this
          costs ~0.05% extra output error)
  apply   fp8 DoubleRow matmuls, lhsT = [bv(6) | ones...] -> psa rows 0..6
          = raw outs + denominator, rows 32..38 = denominator x7
  norm    copy psa rows 0:39 -> SBUF (ACT/DVE), then ONE Pool divide
          cat = rows 0:7 / rows 32:39  (SBUF-only, so Pool is legal)
  fin     (14 -> 64) bf16 matmul -> PSUM -> ACT/DVE copy -> DMA out
The remainder tile is processed mid-stream with ops sliced to its ~16 real
columns (stale te/cat bytes beyond that are second-use ring buffers, hence
finite, and feed only discarded slots); a full tile runs last with its
h0-side apply/norm/fin pre-issued so the post-last-exp chain is short.
"""

import sys

for _p in ("/opt/trn_rl_repo", "/root/.axon_site/_ro/trn_rl_repo"):
    if _p not in sys.path:
        sys.path.append(_p)

import numpy as np

import bass_rust

import concourse.bass as bass
import concourse.mybir as mybir
from concourse.bass_utils import run_bass_kernel_spmd
from concourse.tile import TileContext
from concourse.vector_clock import ScopedClock, VectorClock

F32 = mybir.dt.float32
BF16 = mybir.dt.bfloat16
FP8 = mybir.dt.float8e4
U8 = mybir.dt.uint8

NP_BF16 = mybir.dt.np(BF16)
NP_FP8 = mybir.dt.np(FP8)

# Problem shapes (hardcoded per the harness contract).
M, B, N, FD, BD, BVD, H = 4096, 8, 512, 128, 128, 6, 64
H2 = H // 2
N_CORES = 8

# fp8e4(v) bits = 8*(log2(v)+7) with mantissa-linear interpolation; the +0.36
# centers the 2^frac-1 vs frac error band; +0.5 assumes truncating f32->u8.
SCHRAUDOLPH_MUL = 1.4426950408889634  # log2(e) (the /8 score scale and the
SCHRAUDOLPH_ADD = 56.5 + 0.36  # x8 bits-per-octave cancel)

# The walrus build in this environment rejects multiple semaphore waits
# on one instruction; carry every wait on its own nop.
_MAX_WAITS = 1


class _ChunkedDrainTileContext(TileContext):
    """The walrus build in this environment rejects >1 semaphore wait on a
    single instruction, which breaks TileContext's final drain (it carries
    one wait per touched proc).  Split those waits across one nop per proc;
    each engine executes serially, so a bare drain afterwards is
    equivalent."""

    _nop_uid = 0

    def _add_instruction(self, inst):
        si = inst.sync_info
        if (
            si is not None
            and si.on_wait
            and len(si.on_wait) > _MAX_WAITS
            and inst.engine != mybir.EngineType.Unassigned
        ):
            waits = list(si.on_wait)
            excess, keep = waits[:-_MAX_WAITS], waits[-_MAX_WAITS:]
            for i in range(0, len(excess), _MAX_WAITS):
                _ChunkedDrainTileContext._nop_uid += 1
                nop = mybir.InstNoOp(
                    name=f"splitw{_ChunkedDrainTileContext._nop_uid}", ins=[], outs=[]
                )
                nop.engine = inst.engine
                nop.sync_info = bass_rust.SyncInfo(
                    on_wait=excess[i : i + _MAX_WAITS], on_update=[]
                )
                super()._add_instruction(nop)
            inst.sync_info = bass_rust.SyncInfo(on_wait=keep, on_update=si.on_update)
        super()._add_instruction(inst)

    def _drain_and_barrier(self, tick_clock, wait_clock):
        nc = self.nc
        g = tick_clock.global_clock
        nprocs = len(g)
        for i in range(nprocs):
            if g[i] > 0:
                vc = VectorClock([0] * nprocs)
                vc.require_at_least(i, g[i])
                nop_inst = nc.sync.nop(nofuse=True, hint=f"drain_wait_p{i}")
                wait_clock.add_sem_waits(nop_inst.ins, ScopedClock({None: vc}))
        nc.sync.drain()
        nc.all_engine_barrier()
        assert self.sems is not None
        popped = nc._tile_sem_poison_stack.pop()
        assert popped is self._sem_poison
        nc.clear_and_free_semaphores(list(self.sems.allocated().values()))
        nc.all_engine_barrier()


def _n_tiles(maxL):
    return max(1, -(-maxL // 128))


def _plan(NT, maxL):
    """Per-tile schedule: processing order, exp engines (h0, h1), psa-copy
    engine, fin-copy engine, exp/psa slice width.  'A' = ACT, 'D' = DVE.
    The remainder tile runs second-to-last (its sliced ops are cheap); the
    last full tile's tail is half-pipelined."""
    wr = maxL - 128 * (NT - 1)
    ew = [128] * NT
    if 0 < wr < 128:
        ew[NT - 1] = min(128, -(-wr // 16) * 16)
    if NT >= 3:
        order = list(range(NT - 2)) + [NT - 1, NT - 2]
    else:
        order = list(range(NT))
    # DVE is locked into the fhat bias-adds + all recip/mul norms (~4.5us),
    # which leaves room for only ~2 full exps there; ACT takes the rest.
    # The remainder tile's (tiny, sliced) exps ride ACT so its apply never
    # waits behind DVE's norm backlog.
    exp_eng = {}
    for t in range(NT):
        exp_eng[(t, 0)] = "A"
        exp_eng[(t, 1)] = "D" if (t in (0, 1) and ew[t] == 128) else "A"
    cp_eng = {t: "D" for t in order}
    fc_eng = {t: "A" for t in order}
    return order, exp_eng, cp_eng, fc_eng, ew


def build_program(maxL, reps=1):
    """Build the per-core SPMD Bass program (identical on all 8 cores)."""
    nc = bass.Bass()
    NT = _n_tiles(maxL)
    TM = 128 * NT
    W = 128
    order, exp_eng, cp_eng, fc_eng, ew = _plan(NT, maxL)
    last_t = order[-1]

    d_pk = nc.declare_dram_parameter("pk", [128, 392], BF16, isOutput=False)
    d_bp = nc.declare_dram_parameter("bp", [BD, N], BF16, isOutput=False)
    d_ft = nc.declare_dram_parameter("ft", [FD, TM], BF16, isOutput=False)
    d_out = nc.declare_dram_parameter("outT", [reps, H, TM], BF16, isOutput=True)

    DR = mybir.MatmulPerfMode.DoubleRow
    Exp = mybir.ActivationFunctionType.Exp
    Copy = mybir.ActivationFunctionType.Copy
    Div = mybir.AluOpType.divide
    Mult = mybir.AluOpType.mult
    Add = mybir.AluOpType.add

    # finA column slot per non-last tile (packed in processing order of
    # sorted tile index so out DMA pieces are contiguous in d_out).
    nonlast = sorted(t for t in order if t != last_t)
    fslot = {t: 128 * i for i, t in enumerate(nonlast)}

    with _ChunkedDrainTileContext(nc) as tc, nc.allow_low_precision(
        reason="bf16/fp8 rounding of fp32 data"
    ):
        with (
            tc.tile_pool(name="const", bufs=1) as cp,
            tc.tile_pool(name="work", bufs=2) as wkp,
            tc.tile_pool(name="te", bufs=6) as tep,
            tc.tile_pool(name="ps_s", bufs=4, space="PSUM") as pss,
            tc.tile_pool(name="ps_a", bufs=2, space="PSUM") as psap,
        ):
            # Preload the ACT Exp table at t=0 (a real first Exp would pay
            # the ~1.3us table load on the critical path).
            t_warm = cp.tile([1, 1], F32, tag="warm")
            nc.vector.memset(t_warm[:], 0.0)
            t_warm2 = cp.tile([1, 1], F32, tag="warm2")
            nc.scalar.activation(t_warm2[:], t_warm[:], Exp)

            # Input DMAs: packed weights + ft piece 0 on the SP HWDGE queue;
            # b_pre halves + remaining ft columns on the Pool SWDGE queue
            # (which bypasses the shared HWDGE block).
            t_pk = cp.tile([128, 392], BF16, tag="pk")
            nc.sync.dma_start(t_pk[:], d_pk[:])
            t_ft = cp.tile([FD, TM], BF16, tag="ft")
            nc.sync.dma_start(t_ft[:, 0:128], d_ft[:, 0:128])
            t_bp = cp.tile([BD, N], BF16, tag="bp")
            nc.gpsimd.dma_start(t_bp[:, 0:256], d_bp[:, 0:256])
            nc.gpsimd.dma_start(t_bp[:, 256:512], d_bp[:, 256:512])
            if TM > 128:
                nc.gpsimd.dma_start(t_ft[:, 128:TM], d_ft[:, 128:TM])

            # Views into the packed weights.
            t_wft = t_pk[:, 0:64]
            t_wbt = t_pk[:, 64:128]
            t_wfe = t_pk[0:7, 128:192]
            t_wfg = t_pk[0:7, 192:256]
            t_bf = t_pk[0:64, 256:258].bitcast(F32)
            t_bv = t_pk[:, 258:386].bitcast(FP8).rearrange("p (j c) -> p j c", j=4)

            for rep in range(reps):
                # ---- boundary features: two (128 -> 64, 256) matmuls; the
                # PSUM->SBUF copies ride ACT (first half) and DVE ----
                ps_bh = psap.tile([H, N], F32, tag="finA", bufs=1)
                nc.tensor.matmul(
                    ps_bh[:, 0:256], t_wbt, t_bp[:, 0:256], start=True, stop=True
                )
                nc.tensor.matmul(
                    ps_bh[:, 256:512], t_wbt, t_bp[:, 256:512], start=True, stop=True
                )
                t_bh = wkp.tile([H, N], BF16, tag="bh")
                nc.scalar.copy(t_bh[:, 0:256], ps_bh[:, 0:256])
                nc.scalar.copy(t_bh[:, 256:512], ps_bh[:, 256:512])

                # ---- node features: piece 0 early (gates first scores);
                # all bias-add copies ride DVE ahead of its first exp ----
                t_fh = wkp.tile([H, TM], BF16, tag="fh")
                ps_f0 = psap.tile([H, 128], F32, tag="finB", bufs=1)
                nc.tensor.matmul(ps_f0[:], t_wft, t_ft[:, 0:128], start=True, stop=True)
                nc.vector.tensor_scalar_add(t_fh[:, 0:128], ps_f0[:], t_bf)
                ps_fr = []
                fr_pieces = []
                for i in range(1, NT, 2):
                    j = min(i + 2, NT)
                    p = psap.tile([H, 128 * (j - i)], F32, tag="a")
                    nc.tensor.matmul(
                        p[:], t_wft, t_ft[:, 128 * i : 128 * j],
                        start=True, stop=True,
                    )
                    ps_fr.append(p)
                    fr_pieces.append((i, j))
                # Bias-adds for the remaining pieces go on DVE ahead of its
                # first exp (they are ready long before the exp's scores).
                for p, (i, j) in zip(ps_fr, fr_pieces):
                    nc.vector.tensor_scalar_add(
                        t_fh[:, 128 * i : 128 * j], p[:], t_bf
                    )

                def bh_ap(hi, j):  # noqa: B023
                    return t_bh[H2 * hi : H2 * hi + H2, 128 * j : 128 * (j + 1)]

                def fh_ap(hi, t):  # noqa: B023
                    return t_fh[H2 * hi : H2 * hi + H2, 128 * t : 128 * (t + 1)]

                ps_finA = psap.tile(
                    [H, 128 * max(1, NT - 1)], F32, tag="finA", bufs=1
                )
                ps_finB = psap.tile([H, 128], F32, tag="finB", bufs=1)
                t_fout = wkp.tile([H, TM], BF16, tag="fout")

                te_tiles = {}
                psa_tiles = {}
                sb_tiles = {}
                cat_tiles = {}

                def emit_scores(t, hi):  # noqa: B023
                    ps_s = pss.tile([128, 4, W], F32, tag="s")
                    for j in range(4):
                        nc.tensor.matmul(
                            ps_s[:, j, :], bh_ap(hi, j), fh_ap(hi, t),
                            start=True, stop=True,
                        )
                    return ps_s

                def emit_exp(t, hi, ps_s):  # noqa: B023
                    te = tep.tile([128, 4, W], FP8, tag="te")
                    e = ew[t]
                    if exp_eng[(t, hi)] == "A":
                        nc.scalar.activation(
                            te[:, :, 0:e], ps_s[:, :, 0:e], Exp, scale=0.125
                        )
                    else:
                        nc.vector.tensor_scalar(
                            te[:, :, 0:e].bitcast(U8), ps_s[:, :, 0:e],
                            SCHRAUDOLPH_MUL, SCHRAUDOLPH_ADD, Mult, Add,
                        )
                    te_tiles[(t, hi)] = te

                def emit_apply(t, hi=None):  # noqa: B023
                    # hi=None: both halves; else just that half's 2 DR mms.
                    if t not in psa_tiles:
                        psa_tiles[t] = psap.tile([64, 2, W], F32, tag="a", name=f"psa{t}")
                    t_psa = psa_tiles[t]
                    his = range(2) if hi is None else [hi]
                    for h in his:
                        te = te_tiles[(t, h)]
                        for jp in range(2):
                            nc.tensor.matmul(
                                t_psa[0:64, h, :],
                                t_bv[:, 2 * jp : 2 * jp + 2, :],
                                te[:, 2 * jp : 2 * jp + 2, :],
                                start=(jp == 0), stop=(jp == 1),
                                perf_mode=DR,
                            )

                def emit_norm(t, hi=None):  # noqa: B023
                    # DVE reciprocal of the replicated denominator rows
                    # 32:39 into SBUF, then DVE multiply psa rows 0:7 (the
                    # single legal PSUM operand) by the quadrant-aligned
                    # reciprocal.  hi!=None runs a per-half slice so the
                    # last tile's h0 norm overlaps its h1 exp.
                    t_psa = psa_tiles[t]
                    e = ew[t]
                    if t not in sb_tiles:
                        sb_tiles[t] = wkp.tile([39, 2, W], F32, tag="psb", name=f"psb{t}")
                        cat_tiles[t] = wkp.tile([64, 2, W], BF16, tag="cat", name=f"cat{t}")
                    t_sb = sb_tiles[t]
                    t_cat = cat_tiles[t]
                    his = slice(None) if hi is None else slice(hi, hi + 1)
                    nc.vector.reciprocal(
                        t_sb[32:39, his, 0:e], t_psa[32:39, his, 0:e]
                    )
                    nc.vector.tensor_mul(
                        t_cat[0:7, his, 0:e], t_psa[0:7, his, 0:e],
                        t_sb[32:39, his, 0:e],
                    )

                def emit_fin(t):  # noqa: B023
                    t_cat = cat_tiles[t]
                    if t == last_t:
                        fin = ps_finB[:, 0:W]
                    else:
                        fin = ps_finA[:, fslot[t] : fslot[t] + W]
                    nc.tensor.matmul(
                        fin, t_wfe, t_cat[0:7, 0, :], start=True, stop=False
                    )
                    nc.tensor.matmul(
                        fin, t_wfg, t_cat[0:7, 1, :], start=False, stop=True
                    )

                def emit_fin_copy(t):  # noqa: B023
                    # fin PSUM -> t_fout columns (sliced to real width).
                    e = ew[t]
                    src = (
                        ps_finB[:, 0:e]
                        if t == last_t
                        else ps_finA[:, fslot[t] : fslot[t] + e]
                    )
                    dst = t_fout[:, 128 * t : 128 * t + e]
                    if fc_eng[t] == "A":
                        nc.scalar.copy(dst, src)
                    else:
                        nc.vector.tensor_copy(dst, src)

                # --- main pipeline over processing order ---
                # pos i emits scores+exps(i); apply/norm trail by 1, fin by 2.
                def tail_emit(i):  # noqa: B023
                    if i >= 1:
                        emit_apply(order[i - 1])
                        emit_norm(order[i - 1])
                    if i >= 2:
                        emit_fin(order[i - 2])
                        emit_fin_copy(order[i - 2])

                for i, t in enumerate(order[:-1]):
                    for hi in range(2):
                        ps_s = emit_scores(t, hi)
                        emit_exp(t, hi, ps_s)
                    tail_emit(i)
                # Last tile: h0's exp/apply/norm-copy and even its fin start=
                # half are issued while h1's exp runs, so the post-last-exp
                # chain is apply(h1) -> copy(h1) -> divide -> fin -> copy.
                i = len(order) - 1
                ps_s = emit_scores(last_t, 0)
                emit_exp(last_t, 0, ps_s)
                ps_s = emit_scores(last_t, 1)
                tail_emit(i)
                emit_apply(last_t, hi=0)
                emit_norm(last_t, hi=0)
                emit_exp(last_t, 1, ps_s)
                emit_apply(last_t, hi=1)
                emit_norm(last_t, hi=1)
                if i >= 1:
                    emit_fin(order[i - 1])
                    emit_fin_copy(order[i - 1])
                emit_fin(last_t)
                emit_fin_copy(last_t)

                # --- out DMAs: contiguous runs of non-last tiles on SP,
                # the last tile's piece on Pool SWDGE ---
                runs = []
                for t in nonlast:
                    if runs and runs[-1][1] == t:
                        runs[-1][1] = t + 1
                    else:
                        runs.append([t, t + 1])
                for t0, t1 in runs:
                    e = 128 * (t1 - 1 - t0) + ew[t1 - 1]
                    nc.sync.dma_start(
                        d_out[rep][:, 128 * t0 : 128 * t0 + e],
                        t_fout[:, 128 * t0 : 128 * t0 + e],
                    )
                nc.gpsimd.dma_start(
                    d_out[rep][:, 128 * last_t : 128 * last_t + ew[last_t]],
                    t_fout[:, 128 * last_t : 128 * last_t + ew[last_t]],
                )

    return nc


def stage_inputs(inputs, bounds, maxL):
    """Build the 8 per-core input maps from the full problem inputs."""
    TM = 128 * _n_tiles(maxL)
    f_pre_in = np.asarray(inputs["f_pre_in"], dtype=np.float32)
    b_pre_in = np.asarray(inputs["b_pre_in"], dtype=np.float32)
    bv_in = np.asarray(inputs["bv_in"], dtype=np.float32)
    Wf = np.asarray(inputs["Wf"], dtype=np.float32)
    bf = np.asarray(inputs["bf"], dtype=np.float32)
    Wb = np.asarray(inputs["Wb"], dtype=np.float32)
    Wbv = np.asarray(inputs["Wbv"], dtype=np.float32)
    bbv = np.asarray(inputs["bbv"], dtype=np.float32)
    Wo = np.asarray(inputs["Wo"], dtype=np.float32)
    bo = np.asarray(inputs["bo"], dtype=np.float32)

    # Folded output weights: out = A @ (bv@w_e)/de + Bm @ (bv@w_g)/dg + bo_tot
    A = Wo[:, :H] @ Wbv
    Bm = Wo[:, H:] @ Wbv
    bo_tot = bo + (Wo[:, :H] + Wo[:, H:]) @ bbv

    wpk = np.zeros((128, 256), np.float32)
    wpk[:, 0:64] = Wf.T
    wpk[:, 64:128] = Wb.T
    # final lhsT: rows 0-5 = A cols, 6 = bo_tot, same for Bm (geo)
    wpk[0:6, 128:192] = A.T
    wpk[6, 128:192] = bo_tot
    wpk[0:6, 192:256] = Bm.T

    in_maps = []
    for c in range(N_CORES):
        s, e = int(bounds[c]), int(bounds[c + 1])
        L = e - s
        ft = np.zeros((FD, TM), np.float32)
        ft[:, :L] = f_pre_in[s:e].T
        bv8 = np.zeros((128, 4, 64), np.float32)
        for j in range(4):
            bv8[:, j, 0:BVD] = bv_in[c, :, 128 * j : 128 * (j + 1)].T
        bv8[:, :, 6] = 1.0       # -> psa row 6 = denominator (cat ones row)
        bv8[:, :, 32:39] = 1.0   # -> psa rows 32-38 = denominator block
        # Packed consts: wpk bf16 | bf f32 (rows 0:64) | bv8 fp8, as bytes.
        pk = np.zeros((128, 784), np.uint8)
        pk[:, 0:512] = wpk.astype(NP_BF16).view(np.uint8)
        pk[0:64, 512:516] = bf.astype(np.float32).view(np.uint8).reshape(64, 4)
        pk[:, 516:772] = bv8.astype(NP_FP8).reshape(128, 256).view(np.uint8)
        in_maps.append(
            {
                "pk": pk.view(NP_BF16),
                "ft": ft.astype(NP_BF16),
                "bp": b_pre_in[c].astype(NP_BF16),
            }
        )
    return in_maps


def unstage_output(results, bounds, maxL):
    out = np.zeros((M, H), np.float32)
    for c in range(N_CORES):
        s, e = int(bounds[c]), int(bounds[c + 1])
        L = e - s
        outT = results[c]["outT"][0]
        out[s:e] = outT[:, :L].T.astype(np.float32)
    return out


_NC_CACHE = {}


def _program(maxL, reps=1):
    key = (maxL, reps)
    if key not in _NC_CACHE:
        _NC_CACHE[key] = build_program(maxL, reps=reps)
    return _NC_CACHE[key]


def kernel(**inputs):
    assert np.asarray(inputs["f_pre_in"]).shape == (M, FD)
    batch = np.asarray(inputs["f_pre_batch"]).astype(np.int64)
    bounds = np.searchsorted(batch, np.arange(B + 1))
    maxL = int(np.diff(bounds).max())
    in_maps = stage_inputs(inputs, bounds, maxL)
    nc = _program(maxL)
    res = run_bass_kernel_spmd(nc, in_maps, core_ids=list(range(N_CORES)))
    return unstage_output(res.results, bounds, maxL)


if __name__ == "__main__":
    rng = np.random.default_rng(0)
    demo = {
        "f_pre_in": rng.standard_normal((M, FD), dtype=np.float32),
        "f_pre_batch": np.sort(rng.integers(0, B, size=M)),
        "b_pre_in": rng.standard_normal((B, BD, N), dtype=np.float32),
        "bv_in": rng.standard_normal((B, BVD, N), dtype=np.float32),
        "Wf": rng.standard_normal((H, FD), dtype=np.float32) * 0.05,
        "bf": rng.standard_normal(H, dtype=np.float32) * 0.05,
        "Wb": rng.standard_normal((H, BD), dtype=np.float32) * 0.05,
        "bb": rng.standard_normal(H, dtype=np.float32) * 0.05,
        "Wbv": rng.standard_normal((H, BVD), dtype=np.float32) * 0.05,
        "bbv": rng.standard_normal(H, dtype=np.float32) * 0.05,
        "Wo": rng.standard_normal((H, 2 * H), dtype=np.float32) * 0.05,
        "bo": rng.standard_normal(H, dtype=np.float32) * 0.05,
    }
    out = kernel(**demo)
    print("kernel output", out.shape, out.dtype, float(np.abs(out).mean()))
